# Initial kernel scaffold
#
"""MoE BaseLayer gate (nn_BaseLayerGate) for 8 Trainium2 NeuronCores.

Strategy
--------
Token-parallel: the 65536x1024 `features` matrix is sharded row-wise over the
8 cores. Each core computes its local gate affinity block
    affin_local[8192, 64] = features_local @ wg_weight.T
with a raw-Bass kernel (the memory-bound part: 32 MB of features per core
streamed once from HBM). The matmul is arranged to be bit-identical to the
XLA-on-trn2 fp32 lowering: the features chunk [128k x 128tok] is the
stationary PE operand, the gate weight chunk [128k x 64] is the moving
operand, and the 8 k-chunks accumulate in ascending order in fp32 PSUM.

The sequential balanced assignment (greedy per-expert top-C with masking, a
stand-in for the auction C++ routine that runs on CPU in the original model)
is data-dependent scalar logic over the tiny [65536, 64] affinity matrix and
runs on the host, exactly replicating jax.lax.top_k semantics
(value-descending, ties by lower index first).

Outputs (matching the reference):
  sort_by_expert [65536] int32, input_splits/output_splits [64] int32 (all C),
  routing_probs [65536, 1] f32 (straight-through: exactly 1.0 in forward).
"""
import sys

sys.path.insert(0, "/opt/trn_rl_repo")
import numpy as np
import concourse.bass as bass
import concourse.mybir as mybir
from concourse.bass_utils import run_bass_kernel_spmd

N_CORES = 8
N_TOK = 65536
D = 1024
E = 64
C = N_TOK // E  # 1024 tokens per expert
TOK_PER_CORE = N_TOK // N_CORES  # 8192
KCH = D // 128  # 8 k-chunks
TBLK = 512  # tokens per pipeline block
SLOTS = 3

_program_cache = {}


def _build_program(tblk=TBLK, slots=SLOTS):
    key = (tblk, slots)
    if key in _program_cache:
        return _program_cache[key]
    nblk = TOK_PER_CORE // tblk
    ntt = tblk // 128
    S = slots
    f32 = mybir.dt.float32
    nc = bass.Bass("TRN2", target_bir_lowering=False, debug=False)
    featT = nc.dram_tensor("featT", [D, TOK_PER_CORE], f32, kind="ExternalInput")
    wgT = nc.dram_tensor("wgT", [D, E], f32, kind="ExternalInput")
    affin = nc.dram_tensor("affin", [TOK_PER_CORE, E], f32, kind="ExternalOutput")

    featT_r = featT.ap().rearrange("(kc p) t -> p kc t", p=128)
    wgT_r = wgT.ap().rearrange("(kc p) e -> p kc e", p=128)
    affin_r = affin.ap().rearrange("(t p) e -> p t e", p=128)

    # PSUM slots padded to whole 2KB banks so concurrent PE-write/DVE-read of
    # different slots never share a bank.
    ps_stride = ((ntt * E + 511) // 512) * 512

    with (
        nc.sbuf_tensor("wg_sb", [128, KCH * E], f32) as wg_sb,
        nc.sbuf_tensor("ft_sb", [128, S * KCH * tblk], f32) as ft_sb,
        nc.sbuf_tensor("ot_sb", [128, S * ntt * E], f32) as ot_sb,
        nc.psum_tensor("ps_all", [128, S * ps_stride], f32) as ps_all,
        nc.semaphore("wg_sem") as wg_sem,
        nc.semaphore("mm_sem") as mm_sem,
        nc.semaphore("cp_sem") as cp_sem,
    ):
        ld_sems = [nc.alloc_semaphore(f"ld{s}_sem") for s in range(S)]
        st_sems = [nc.alloc_semaphore(f"st{s}_sem") for s in range(S)]
        wg_ap = wg_sb.ap()
        ft_ap = ft_sb.ap()
        ot_ap = ot_sb.ap()
        ps_ap = ps_all.ap()

        def ft_chunk(slot, k):
            off = (slot * KCH + k) * tblk
            return ft_ap[:, off:off + tblk]

        def wg_chunk(k):
            return wg_ap[:, k * E:(k + 1) * E]

        def ps_slot(slot):
            return ps_ap[:, slot * ps_stride:slot * ps_stride + ntt * E]

        def ot_slot(slot):
            return ot_ap[:, slot * ntt * E:(slot + 1) * ntt * E]

        with nc.Block() as block:

            @block.sync
            def _(sync):
                for k in range(KCH):
                    sync.dma_start(wg_chunk(k), wgT_r[:, k, :]).then_inc(wg_sem, 16)
                for b in range(nblk):
                    slot = b % S
                    if b >= S:
                        # PE finished reading this ft slot (block b-S)
                        sync.wait_ge(mm_sem, b - S + 1)
                    for k in range(KCH):
                        sync.dma_start(
                            ft_chunk(slot, k),
                            featT_r[:, k, b * tblk:(b + 1) * tblk],
                        ).then_inc(ld_sems[slot], 16)

            @block.tensor
            def _(tensor):
                tensor.wait_ge(wg_sem, 16 * KCH)
                for b in range(nblk):
                    slot = b % S
                    ps = ps_slot(slot)
                    tensor.wait_ge(ld_sems[slot], 16 * KCH * (b // S + 1))
                    if b >= S:
                        # DVE drained this psum slot (block b-S)
                        tensor.wait_ge(cp_sem, b - S + 1)
                    for tt in range(ntt):
                        for k in range(KCH):
                            mm = tensor.matmul(
                                ps[:, tt * E:(tt + 1) * E],
                                ft_chunk(slot, k)[:, tt * 128:(tt + 1) * 128],
                                wg_chunk(k),
                                start=(k == 0),
                                stop=(k == KCH - 1),
                            )
                    mm.then_inc(mm_sem)

            @block.vector
            def _(vector):
                for b in range(nblk):
                    slot = b % S
                    vector.wait_ge(mm_sem, b + 1)
                    if b >= S:
                        # store drained this ot slot (block b-S)
                        vector.wait_ge(st_sems[slot], 16 * (b // S))
                    vector.tensor_copy(ot_slot(slot), ps_slot(slot)).then_inc(cp_sem)

            @block.gpsimd
            def _(gp):
                for b in range(nblk):
                    slot = b % S
                    gp.wait_ge(cp_sem, b + 1)
                    ot3 = ot_slot(slot).rearrange("p (tt e) -> p tt e", tt=ntt)
                    gp.dma_start(
                        affin_r[:, b * (tblk // 128):(b + 1) * (tblk // 128), :], ot3
                    ).then_inc(st_sems[slot], 16)

    _program_cache[key] = nc
    return nc


def _gate_affinity(feats, wg):
    """affin[65536, 64] = feats @ wg.T on the 8 NeuronCores (bass SPMD)."""
    nc = _build_program()
    wgT = np.ascontiguousarray(wg.T)
    in_maps = [
        {
            "featT": np.ascontiguousarray(
                feats[i * TOK_PER_CORE:(i + 1) * TOK_PER_CORE, :].T
            ),
            "wgT": wgT,
        }
        for i in range(N_CORES)
    ]
    res = run_bass_kernel_spmd(nc, in_maps, list(range(N_CORES)))
    return np.concatenate([res.results[i]["affin"] for i in range(N_CORES)], axis=0)


def _balanced_assignment(affin):
    """Greedy per-expert top-C with masking; identical to the reference's
    scan of jax.lax.top_k over masked columns (ties: lower index first)."""
    N, nE = affin.shape
    cap = N // nE
    # stable descending order per column == lax.top_k tie semantics
    order = np.argsort(-affin, axis=0, kind="stable")
    assigned = np.zeros(N, dtype=bool)
    out = np.empty((nE, cap), np.int32)
    for e in range(nE):
        cand = order[:, e]
        unassigned = cand[~assigned[cand]]
        sel = unassigned[:cap]
        out[e] = sel
        assigned[sel] = True
    return out.reshape(-1)


def kernel(features, wg_weight):
    feats = np.ascontiguousarray(np.asarray(features, dtype=np.float32))
    wg = np.ascontiguousarray(np.asarray(wg_weight, dtype=np.float32))
    assert feats.shape == (N_TOK, D) and wg.shape == (E, D)

    affin = _gate_affinity(feats, wg)

    # reference's isfinite fixup: non-finite -> min of finite scores
    ok = np.isfinite(affin)
    if not ok.all():
        fmin = affin[ok].min() if ok.any() else np.float32(np.inf)
        affin = np.where(ok, affin, fmin).astype(np.float32)

    sort_by_expert = _balanced_assignment(affin)
    input_splits = np.full((E,), C, dtype=np.int32)
    output_splits = input_splits.copy()
    # straight-through gate: p - stop_grad(p) + 1 == exactly 1.0 in forward
    routing_probs = np.ones((N_TOK, 1), dtype=np.float32)
    return sort_by_expert, input_splits, output_splits, routing_probs


# revision 3
# speedup vs baseline: 1.0658x; 1.0658x over previous
"""MoE BaseLayer gate (nn_BaseLayerGate) for 8 Trainium2 NeuronCores.

Strategy
--------
Token-parallel: the 65536x1024 `features` matrix is sharded row-wise over the
8 cores. Each core computes its local gate affinity block
    affin_local[8192, 64] = features_local @ wg_weight.T
with a raw-Bass kernel (the memory-bound part: 32 MB of features per core
streamed once from HBM). The matmul is arranged to be bit-identical to the
XLA-on-trn2 fp32 lowering: the features chunk [128k x 128tok] is the
stationary PE operand, the gate weight chunk [128k x 64] is the moving
operand, and the 8 k-chunks accumulate in ascending order in fp32 PSUM.

The sequential balanced assignment (greedy per-expert top-C with masking, a
stand-in for the auction C++ routine that runs on CPU in the original model)
is data-dependent scalar logic over the tiny [65536, 64] affinity matrix and
runs on the host, exactly replicating jax.lax.top_k semantics
(value-descending, ties by lower index first).

Outputs (matching the reference):
  sort_by_expert [65536] int32, input_splits/output_splits [64] int32 (all C),
  routing_probs [65536, 1] f32 (straight-through: exactly 1.0 in forward).

The Bass program is built from a source string compiled under a fixed
pseudo-filename so the emitted BIR (which embeds source locations) is
byte-identical regardless of the directory this file runs from — keeping the
neuron JIT compile cache warm across working directories.
"""
import sys

sys.path.insert(0, "/opt/trn_rl_repo")
import numpy as np
import concourse.bass as bass
import concourse.mybir as mybir
from concourse.bass_utils import run_bass_kernel_spmd

N_CORES = 8
N_TOK = 65536
D = 1024
E = 64
C = N_TOK // E  # 1024 tokens per expert
TOK_PER_CORE = N_TOK // N_CORES  # 8192
KCH = D // 128  # 8 k-chunks
TBLK = 512  # tokens per pipeline block
SLOTS = 4

_BUILD_SRC = '''
def _build(bass, mybir, D, E, TOK_PER_CORE, KCH, tblk, S):
    nblk = TOK_PER_CORE // tblk
    ntt = tblk // 128
    f32 = mybir.dt.float32
    nc = bass.Bass("TRN2", target_bir_lowering=False, debug=False)
    featT = nc.dram_tensor("featT", [D, TOK_PER_CORE], f32, kind="ExternalInput")
    wgT = nc.dram_tensor("wgT", [D, E], f32, kind="ExternalInput")
    affin = nc.dram_tensor("affin", [TOK_PER_CORE, E], f32, kind="ExternalOutput")

    featT_r = featT.ap().rearrange("(kc p) t -> p kc t", p=128)
    wgT_r = wgT.ap().rearrange("(kc p) e -> p kc e", p=128)
    affin_r = affin.ap().rearrange("(t p) e -> p t e", p=128)

    # PSUM slots padded to whole 2KB banks so concurrent PE-write/DVE-read of
    # different slots never share a bank.
    ps_stride = ((ntt * E + 511) // 512) * 512

    with (
        nc.sbuf_tensor("wg_sb", [128, KCH * E], f32) as wg_sb,
        nc.sbuf_tensor("ft_sb", [128, S * KCH * tblk], f32) as ft_sb,
        nc.sbuf_tensor("ot_sb", [128, S * ntt * E], f32) as ot_sb,
        nc.psum_tensor("ps_all", [128, S * ps_stride], f32) as ps_all,
        nc.semaphore("wg_sem") as wg_sem,
        nc.semaphore("mm_sem") as mm_sem,
        nc.semaphore("cp_sem") as cp_sem,
    ):
        ld_sems = [nc.alloc_semaphore("ld%d_sem" % s) for s in range(S)]
        st_sems = [nc.alloc_semaphore("st%d_sem" % s) for s in range(S)]
        wg_ap = wg_sb.ap()
        ft_ap = ft_sb.ap()
        ot_ap = ot_sb.ap()
        ps_ap = ps_all.ap()

        def ft_chunk(slot, k):
            off = (slot * KCH + k) * tblk
            return ft_ap[:, off:off + tblk]

        def wg_chunk(k):
            return wg_ap[:, k * E:(k + 1) * E]

        def ps_slot(slot):
            return ps_ap[:, slot * ps_stride:slot * ps_stride + ntt * E]

        def ot_slot(slot):
            return ot_ap[:, slot * ntt * E:(slot + 1) * ntt * E]

        with nc.Block() as block:

            @block.sync
            def _(sync):
                for k in range(KCH):
                    sync.dma_start(wg_chunk(k), wgT_r[:, k, :]).then_inc(wg_sem, 16)
                for b in range(nblk):
                    slot = b % S
                    if b >= S:
                        # PE finished reading this ft slot (block b-S)
                        sync.wait_ge(mm_sem, b - S + 1)
                    for k in range(KCH):
                        sync.dma_start(
                            ft_chunk(slot, k),
                            featT_r[:, k, b * tblk:(b + 1) * tblk],
                        ).then_inc(ld_sems[slot], 16)

            @block.tensor
            def _(tensor):
                tensor.wait_ge(wg_sem, 16 * KCH)
                for b in range(nblk):
                    slot = b % S
                    ps = ps_slot(slot)
                    tensor.wait_ge(ld_sems[slot], 16 * KCH * (b // S + 1))
                    if b >= S:
                        # DVE drained this psum slot (block b-S)
                        tensor.wait_ge(cp_sem, b - S + 1)
                    for tt in range(ntt):
                        for k in range(KCH):
                            mm = tensor.matmul(
                                ps[:, tt * E:(tt + 1) * E],
                                ft_chunk(slot, k)[:, tt * 128:(tt + 1) * 128],
                                wg_chunk(k),
                                start=(k == 0),
                                stop=(k == KCH - 1),
                            )
                    mm.then_inc(mm_sem)

            @block.vector
            def _(vector):
                for b in range(nblk):
                    slot = b % S
                    vector.wait_ge(mm_sem, b + 1)
                    if b >= S:
                        # store drained this ot slot (block b-S)
                        vector.wait_ge(st_sems[slot], 16 * (b // S))
                    vector.tensor_copy(ot_slot(slot), ps_slot(slot)).then_inc(cp_sem)

            @block.gpsimd
            def _(gp):
                for b in range(nblk):
                    slot = b % S
                    gp.wait_ge(cp_sem, b + 1)
                    ot3 = ot_slot(slot).rearrange("p (tt e) -> p tt e", tt=ntt)
                    gp.dma_start(
                        affin_r[:, b * ntt:(b + 1) * ntt, :], ot3
                    ).then_inc(st_sems[slot], 16)

    return nc
'''

_program_cache = {}


def _build_program(tblk=TBLK, slots=SLOTS):
    key = (tblk, slots)
    if key not in _program_cache:
        ns = {}
        exec(compile(_BUILD_SRC, "<nn_gate_bass>", "exec"), ns)
        _program_cache[key] = ns["_build"](
            bass, mybir, D, E, TOK_PER_CORE, KCH, tblk, slots
        )
    return _program_cache[key]


def _gate_affinity(feats, wg):
    """affin[65536, 64] = feats @ wg.T on the 8 NeuronCores (bass SPMD)."""
    nc = _build_program()
    wgT = np.ascontiguousarray(wg.T)
    in_maps = [
        {
            "featT": np.ascontiguousarray(
                feats[i * TOK_PER_CORE:(i + 1) * TOK_PER_CORE, :].T
            ),
            "wgT": wgT,
        }
        for i in range(N_CORES)
    ]
    res = run_bass_kernel_spmd(nc, in_maps, list(range(N_CORES)))
    return np.concatenate([res.results[i]["affin"] for i in range(N_CORES)], axis=0)


def _balanced_assignment(affin):
    """Greedy per-expert top-C with masking; identical to the reference's
    scan of jax.lax.top_k over masked columns (ties: lower index first)."""
    N, nE = affin.shape
    cap = N // nE
    # stable descending order per column == lax.top_k tie semantics
    order = np.argsort(-affin, axis=0, kind="stable")
    assigned = np.zeros(N, dtype=bool)
    out = np.empty((nE, cap), np.int32)
    for e in range(nE):
        cand = order[:, e]
        unassigned = cand[~assigned[cand]]
        sel = unassigned[:cap]
        out[e] = sel
        assigned[sel] = True
    return out.reshape(-1)


def kernel(features, wg_weight):
    feats = np.ascontiguousarray(np.asarray(features, dtype=np.float32))
    wg = np.ascontiguousarray(np.asarray(wg_weight, dtype=np.float32))
    assert feats.shape == (N_TOK, D) and wg.shape == (E, D)

    affin = _gate_affinity(feats, wg)

    # reference's isfinite fixup: non-finite -> min of finite scores
    ok = np.isfinite(affin)
    if not ok.all():
        fmin = affin[ok].min() if ok.any() else np.float32(np.inf)
        affin = np.where(ok, affin, fmin).astype(np.float32)

    sort_by_expert = _balanced_assignment(affin)
    input_splits = np.full((E,), C, dtype=np.int32)
    output_splits = input_splits.copy()
    # straight-through gate: p - stop_grad(p) + 1 == exactly 1.0 in forward
    routing_probs = np.ones((N_TOK, 1), dtype=np.float32)
    return sort_by_expert, input_splits, output_splits, routing_probs
